# revision 6
# baseline (speedup 1.0000x reference)
"""BasicGNNConv on 8 TRN2 NeuronCores (Bass/Tile).

Math (reference):
    h   = node_feat @ Wn + bn                    # [N, 128]
    e   = edge_feat @ We + be                    # [E, 128]
    m   = h[src] + e
    agg = segment_sum(m, dst) / max(deg, 1)
    out = concat([h, agg]) @ Wc + bc

Linearity rewrite (eliminates all per-edge matmuls; biases folded):
    ht   = node_feat @ Wn                        # no bias
    S'   = onehot(dst) * rcol[dst]               # mean folded into the one-hot
    aggT = (S'h gathered-sum)T + We.T @ (S'ef sum)T          # [feat, slot]
    outT = Wc1.T @ htT_own + Wc2.T @ aggT + bnbeWc2 (x) mcol + bias0 (x) 1

Sharding: edges are assigned to the core that owns their dst node range
(5000 nodes/core) -> per-core segment sums are complete, no collective needed.

Per-core device pipeline:
  A.  ht (fp16) for all 40000 nodes in 2048-node chunks, written to two
      partition-major HBM tables (node n -> row (n%128)*TCOLS + n//128) so
      both the chunked writes and the per-edge gathers use efficient
      descriptors; gather indices are host-remapped to this layout.
  A2. ht.T for the core's own 5000 nodes (fp16, kept in SBUF).
  B.  Edge stream grouped by (superblock of 2 dst-blocks, src-half, block):
      gather ht[src] rows in up-to-24-tile SWDGE calls (64KB descriptor
      carveout -> 4096-desc rings), build the rcol-scaled one-hot S' with a
      single DVE tensor_scalar (is_equal then mult), and accumulate the
      TRANSPOSED segment sums acc_hT/acc_eT = [feat, slot] into one shared
      PSUM bank per block (lhsT = data tile, rhs = S').  The epilogue applies
      We to acc_eT (no transposes needed), then emits the output transposed;
      the host un-transposes.
"""
import numpy as np

import concourse.bacc as bacc
import concourse.mybir as mybir
import concourse.tile as tile
from concourse.tile_rust import add_dep_helper
from concourse.bass_utils import run_bass_kernel_spmd

N = 40000
E = 640000
D = 128          # OUT_DIM == EDGE_DIM
ND = 256         # NODE_DIM
C = 8            # cores
NPC = N // C     # 5000 nodes per core
BLK = 125        # nodes per dst block
NB = NPC // BLK  # 40 blocks per core
SBLK = 2         # blocks per superblock (PSUM-bounded)
NSB = NB // SBLK
NLO = 20480      # nodes in the lo gather table (10 phase-A chunks)
TLO = NLO // 128          # 160 t-columns
NHICAP = 19584            # 153 * 128 (capacity; real nodes 19520)
THI = NHICAP // 128       # 153
CH = 2048        # phase A chunk (nodes)
NCH = (N + CH - 1) // CH  # 20 (last chunk 1088 nodes)
CH2 = 500        # phase A2 chunk (own nodes)
PAD_COL = 127    # trash column in the 128-wide S window (>= BLK)
GMAX = 8         # tiles per dma_gather call
NQ = 4           # SWDGE queues
SCRATCH = 16384  # dynamic DMA scratch (default)

LAST_EXEC_NS = None
LAST_RESULTS = None

f16 = np.float16


def _wrap_idx16(arr):
    """[L] -> [128, L//16] int16 wrapped layout (pos i at [i%16, i//16]),
    replicated across the 8 GPSIMD core partition groups."""
    w = arr.astype(np.int16).reshape(-1, 16).T
    return np.ascontiguousarray(np.tile(w, (8, 1)))


def _build_graph(T_list):
    nc = bacc.Bacc(
        None, target_bir_lowering=False, debug=False,
        num_swdge_queues=NQ, dynamic_dma_scratch_size=SCRATCH,
    )
    f32, i16, fh = mybir.dt.float32, mybir.dt.int16, mybir.dt.float16

    T_tot = sum(tl + th for tl, th in T_list)
    L = T_tot * 128

    nfT_p = nc.declare_dram_parameter("nfT", [ND, N], fh, isOutput=False)
    nfTo_p = nc.declare_dram_parameter("nfTo", [ND, NPC], fh, isOutput=False)
    Wn_p = nc.declare_dram_parameter("Wn16", [ND, D], fh, isOutput=False)
    We_p = nc.declare_dram_parameter("We16", [D, D], fh, isOutput=False)
    Wc1_p = nc.declare_dram_parameter("Wc116", [D, D], fh, isOutput=False)
    Wc2_p = nc.declare_dram_parameter("Wc216", [D, D], fh, isOutput=False)
    L2_p = nc.declare_dram_parameter("L2", [2, D], fh, isOutput=False)
    R2_p = nc.declare_dram_parameter("R2", [2, NB * 128], fh, isOutput=False)
    gidx_p = nc.declare_dram_parameter("gidx", [128, L // 16], i16, isOutput=False)
    dstf_p = nc.declare_dram_parameter("dstf", [128, T_tot], f32, isOutput=False)
    rcolE_p = nc.declare_dram_parameter("rcolE", [128, T_tot], f32, isOutput=False)
    ef_p = nc.declare_dram_parameter("ef", [128, T_tot, D], fh, isOutput=False)
    outT_p = nc.declare_dram_parameter("outT", [D, NPC], f32, isOutput=True)

    htab_lo = nc.dram_tensor("htab_lo", [NLO, D], fh)
    htab_hi = nc.dram_tensor("htab_hi", [NHICAP, D], fh)

    with tile.TileContext(nc) as tc:
        with (
            tc.tile_pool(name="const", bufs=1) as cpool,
            tc.tile_pool(name="tabs", bufs=1) as tpool,
        ):
            # ---- constants / weights in SBUF ----
            iota_i = cpool.tile([128, 128], mybir.dt.int32)
            nc.gpsimd.iota(iota_i[:], pattern=[[1, 128]], base=0, channel_multiplier=0)
            iota16 = cpool.tile([128, 128], fh)
            nc.vector.tensor_copy(iota16[:], iota_i[:])

            Wn_sb = cpool.tile([128, ND // 128, D], fh)
            nc.sync.dma_start(out=Wn_sb[:], in_=Wn_p[:].rearrange("(k p) d -> p k d", p=128))
            We_sb = cpool.tile([128, D], fh)
            nc.sync.dma_start(out=We_sb[:], in_=We_p[:])
            Wc1_sb = cpool.tile([128, D], fh)
            nc.sync.dma_start(out=Wc1_sb[:], in_=Wc1_p[:])
            Wc2_sb = cpool.tile([128, D], fh)
            nc.sync.dma_start(out=Wc2_sb[:], in_=Wc2_p[:])
            L2_sb = cpool.tile([2, D], fh)
            nc.sync.dma_start(out=L2_sb[:], in_=L2_p[:])
            R2_sb = cpool.tile([2, NB, 128], fh)
            nc.sync.dma_start(out=R2_sb[:], in_=R2_p[:].rearrange("p (b j) -> p b j", j=128))

            gidx_sb = cpool.tile([128, L // 16], i16)
            nc.sync.dma_start(out=gidx_sb[:], in_=gidx_p[:])
            dstf_sb = cpool.tile([128, T_tot], f32)
            nc.sync.dma_start(out=dstf_sb[:], in_=dstf_p[:])
            rcolE_sb = cpool.tile([128, T_tot], f32)
            nc.sync.dma_start(out=rcolE_sb[:], in_=rcolE_p[:])

            hownT = tpool.tile([128, NB, BLK], fh)  # ht.T of own nodes

            # ---- Phase A: ht (fp16) -> partition-major htab tables ----
            with (
                tc.tile_pool(name="phA", bufs=3) as apool,
                tc.tile_pool(name="psA", bufs=2, space="PSUM") as apsum,
            ):
                last_htab_w = None
                for ci in range(NCH):
                    n0 = ci * CH
                    P = min(CH, N - n0)
                    nsub = (P + 127) // 128
                    nf_t = apool.tile([128, 2, CH], fh, tag="nf")
                    nc.sync.dma_start(
                        out=nf_t[:, :, :P],
                        in_=nfT_p[:, n0 : n0 + P].rearrange("(k p) n -> p k n", p=128),
                    )
                    hb = apool.tile([128, CH // 128, D], fh, tag="hb")
                    for g0 in range(0, nsub, 4):
                        gw = min(4, nsub - g0)
                        ps = apsum.tile([128, 4, D], f32, tag="psA")
                        for s in range(g0, g0 + gw):
                            sp = min(128, P - s * 128)
                            for k in range(2):
                                nc.tensor.matmul(
                                    ps[:sp, s - g0, :],
                                    lhsT=nf_t[:, k, s * 128 : s * 128 + sp],
                                    rhs=Wn_sb[:, k, :],
                                    start=(k == 0),
                                    stop=(k == 1),
                                )
                        nc.scalar.activation(
                            hb[:, g0 : g0 + gw, :], ps[:, :gw, :],
                            mybir.ActivationFunctionType.Copy,
                        )
                    if ci < NLO // CH:
                        dst_ap = htab_lo[:].rearrange("(p t) d -> p t d", p=128)[
                            :, ci * (CH // 128) : ci * (CH // 128) + nsub, :
                        ]
                    else:
                        t0 = (ci - NLO // CH) * (CH // 128)
                        dst_ap = htab_hi[:].rearrange("(p t) d -> p t d", p=128)[
                            :, t0 : t0 + nsub, :
                        ]
                    last_htab_w = nc.sync.dma_start(out=dst_ap, in_=hb[:, :nsub, :])

                # ---- Phase A2: ht.T of own nodes (fp16, transposed layout) ----
                for ci in range(NPC // CH2):
                    n0 = ci * CH2
                    nfo = apool.tile([128, 2, CH2], fh, tag="nfo")
                    d = nc.sync.dma_start(
                        out=nfo[:],
                        in_=nfTo_p[:, n0 : n0 + CH2].rearrange("(k p) n -> p k n", p=128),
                    )
                    add_dep_helper(d.ins, last_htab_w.ins, reason="defer A2 dma past htab")
                    ps2 = apsum.tile([128, 4, BLK], f32, tag="psA2")
                    for k in range(2):
                        nc.tensor.matmul(
                            ps2[:],
                            lhsT=Wn_sb[:, k, :],
                            rhs=nfo[:, k, :],
                            start=(k == 0),
                            stop=(k == 1),
                        )
                    nc.scalar.activation(
                        hownT[:, ci * 4 : ci * 4 + 4, :], ps2[:],
                        mybir.ActivationFunctionType.Copy,
                    )

            # ---- Phase B: edge stream + per-block combine epilogue ----
            sb_T = []
            for sb in range(NSB):
                blocks = [sb * SBLK + j for j in range(SBLK)]
                tlo = sum(T_list[b][0] for b in blocks)
                thi = sum(T_list[b][1] for b in blocks)
                sb_T.append((tlo, thi))
            TSBMAX = max(tl + th for tl, th in sb_T)

            with (
                tc.tile_pool(name="phB", bufs=2) as bpool,
                tc.tile_pool(name="phS", bufs=6) as spool,
                tc.tile_pool(name="phC", bufs=2) as cpl,
                tc.tile_pool(name="phO", bufs=2) as opool,
                tc.tile_pool(name="psB", bufs=2, space="PSUM") as bpsum,
            ):
                goff = 0
                qi = 0
                for sb in range(NSB):
                    blocks = [sb * SBLK + j for j in range(SBLK)]
                    Tlo_sb, Thi_sb = sb_T[sb]
                    Tsb = Tlo_sb + Thi_sb
                    eft = bpool.tile([128, TSBMAX, D], fh, tag="eft")
                    nc.sync.dma_start(out=eft[:, :Tsb, :], in_=ef_p[:, goff : goff + Tsb, :])
                    gl = bpool.tile([128, TSBMAX, D], fh, tag="gl")
                    for c0 in range(0, Tlo_sb, GMAX):
                        ch = min(GMAX, Tlo_sb - c0)
                        nc.gpsimd.dma_gather(
                            gl[:, c0 : c0 + ch, :],
                            htab_lo[:],
                            gidx_sb[:, (goff + c0) * 8 : (goff + c0 + ch) * 8],
                            ch * 128, ch * 128, D,
                            queue_num=qi % NQ,
                        )
                        qi += 1
                    for c0 in range(0, Thi_sb, GMAX):
                        ch = min(GMAX, Thi_sb - c0)
                        nc.gpsimd.dma_gather(
                            gl[:, Tlo_sb + c0 : Tlo_sb + c0 + ch, :],
                            htab_hi[:],
                            gidx_sb[:, (goff + Tlo_sb + c0) * 8 : (goff + Tlo_sb + c0 + ch) * 8],
                            ch * 128, ch * 128, D,
                            queue_num=qi % NQ,
                        )
                        qi += 1

                    accs = {}
                    started = {}
                    for j, b in enumerate(blocks):
                        accs[b] = bpsum.tile(
                            [128, 256], f32, tag=f"acc{j}", name=f"acc{j}_{sb}"
                        )
                        started[b] = False
                    trel = 0
                    for h in range(2):
                        for b in blocks:
                            tcount = T_list[b][h]
                            for u in range(tcount):
                                gt = goff + trel
                                S = spool.tile([128, 128], fh, tag="S")
                                nc.vector.tensor_scalar(
                                    out=S[:], in0=iota16[:],
                                    scalar1=dstf_sb[:, gt : gt + 1],
                                    scalar2=rcolE_sb[:, gt : gt + 1],
                                    op0=mybir.AluOpType.is_equal,
                                    op1=mybir.AluOpType.mult,
                                )
                                acc = accs[b]
                                first = not started[b]
                                started[b] = True
                                last = (u == tcount - 1) and (
                                    h == 1 or T_list[b][1] == 0
                                )
                                # acc_hT[f, slot] += gl_t.T @ S'
                                nc.tensor.matmul(
                                    acc[:, 0:128], lhsT=gl[:, trel, :], rhs=S[:],
                                    start=first, stop=False, skip_group_check=True,
                                )
                                # acc_eT[f, slot] += eft_t.T @ S'
                                # start=False always: MM_h's start=True above
                                # cleared the whole bank's has_written bits, so
                                # this group's first write is already "fresh";
                                # a second start=True would wipe MM_h's tile.
                                nc.tensor.matmul(
                                    acc[:, 128:256], lhsT=eft[:, trel, :], rhs=S[:],
                                    start=False, stop=last, skip_group_check=True,
                                )
                                trel += 1
                    goff += Tsb

                    # ---- per-block epilogue: combine (all transposed) ----
                    osb = opool.tile([128, SBLK, BLK], f32, tag="osb")
                    for j, b in enumerate(blocks):
                        acc = accs[b]
                        SefT = cpl.tile([128, BLK], fh, tag="SefT")
                        nc.scalar.activation(
                            SefT[:], acc[:, 128 : 128 + BLK],
                            mybir.ActivationFunctionType.Copy,
                        )
                        # acc_hT[:, :125] += We.T @ SefT  (same open group)
                        nc.tensor.matmul(
                            acc[:, 0:BLK], lhsT=We_sb[:], rhs=SefT[:],
                            start=False, stop=True, skip_group_check=True,
                        )
                        aggT = cpl.tile([128, BLK], fh, tag="aggT")
                        nc.scalar.activation(
                            aggT[:], acc[:, 0:BLK],
                            mybir.ActivationFunctionType.Copy,
                        )
                        poT = bpsum.tile([128, BLK], f32, tag="poT")
                        nc.tensor.matmul(poT[:], lhsT=Wc1_sb[:], rhs=hownT[:, b, :],
                                         start=True, stop=False)
                        nc.tensor.matmul(poT[:], lhsT=Wc2_sb[:], rhs=aggT[:],
                                         start=False, stop=False)
                        nc.tensor.matmul(poT[:], lhsT=L2_sb[:], rhs=R2_sb[:, b, :BLK],
                                         start=False, stop=True)
                        nc.vector.tensor_copy(osb[:, j, :], poT[:])
                    nc.sync.dma_start(
                        out=outT_p[:, sb * SBLK * BLK : (sb + 1) * SBLK * BLK],
                        in_=osb[:],
                    )

    nc.finalize()
    return nc


def kernel(node_feat, edge_feat, Wn, bn, We, be, Wc, bc, src, dst):
    global LAST_EXEC_NS, LAST_RESULTS
    node_feat = np.asarray(node_feat, np.float32)
    edge_feat = np.asarray(edge_feat, np.float32)
    Wn = np.asarray(Wn, np.float32)
    bn = np.asarray(bn, np.float32)
    We = np.asarray(We, np.float32)
    be = np.asarray(be, np.float32)
    Wc = np.asarray(Wc, np.float32)
    bc = np.asarray(bc, np.float32)
    src = np.asarray(src).astype(np.int64)
    dst = np.asarray(dst).astype(np.int64)

    # ---- host-side edge sharding / ordering ----
    cid = dst // NPC
    rel = dst - cid * NPC
    blk = rel // BLK
    dl = (rel - blk * BLK).astype(np.int64)
    sbi = blk // SBLK
    jin = blk - sbi * SBLK
    half = (src >= NLO).astype(np.int64)
    # stream order: (core, superblock, half, block-within-superblock)
    group = ((cid * NSB + sbi) * 2 + half) * SBLK + jin
    order = np.argsort(group, kind="stable")
    counts = np.bincount(group, minlength=C * NSB * 2 * SBLK).reshape(C, NSB, 2, SBLK)
    tcnt = (counts + 127) // 128
    Tmax = tcnt.max(axis=0)                      # [NSB, 2, SBLK]
    T_list = [
        (int(Tmax[b // SBLK, 0, b % SBLK]), int(Tmax[b // SBLK, 1, b % SBLK]))
        for b in range(NB)
    ]
    T_tot = int(Tmax.sum())
    L = T_tot * 128

    deg = np.bincount(dst, minlength=N).astype(np.float32)
    rcol_all = 1.0 / np.maximum(deg, 1.0)
    mcol_all = np.minimum(deg, 1.0)

    # remapped gather row index (partition-major table layout)
    gmap = np.where(
        src < NLO,
        (src % 128) * TLO + src // 128,
        ((src - NLO) % 128) * THI + (src - NLO) // 128,
    ).astype(np.int16)

    ef_h = edge_feat.astype(f16)
    rcolE_edge = rcol_all[dst].astype(np.float32)

    # slot offsets in stream order
    gstart = np.zeros(C * NSB * 2 * SBLK + 1, np.int64)
    np.cumsum(counts.ravel(), out=gstart[1:])
    slot_off = np.zeros(NSB * 2 * SBLK + 1, np.int64)
    np.cumsum(Tmax.ravel() * 128, out=slot_off[1:])

    bnbeWc2 = (bn + be) @ Wc[D:]
    bias0 = bn @ Wc[:D] + bc
    L2 = np.stack([bnbeWc2, bias0]).astype(f16)

    in_maps = []
    shared = {
        "nfT": np.ascontiguousarray(node_feat.T.astype(f16)),
        "Wn16": Wn.astype(f16),
        "We16": We.astype(f16),
        "Wc116": np.ascontiguousarray(Wc[:D]).astype(f16),
        "Wc216": np.ascontiguousarray(Wc[D:]).astype(f16),
        "L2": L2,
    }
    for c in range(C):
        gidx = np.zeros(L, np.int16)
        dstl = np.full(L, PAD_COL, np.float32)
        rcole = np.zeros(L, np.float32)
        eids = np.full(L, -1, np.int64)
        for g_local in range(NSB * 2 * SBLK):
            g = c * (NSB * 2 * SBLK) + g_local
            n = counts.ravel()[g]
            s0 = gstart[g]
            o0 = slot_off[g_local]
            ed = order[s0 : s0 + n]
            gidx[o0 : o0 + n] = gmap[ed]
            dstl[o0 : o0 + n] = dl[ed].astype(np.float32)
            rcole[o0 : o0 + n] = rcolE_edge[ed]
            eids[o0 : o0 + n] = ed
        ef_rows = np.zeros((L, D), f16)
        real = eids >= 0
        ef_rows[real] = ef_h[eids[real]]
        R2 = np.zeros((2, NB, 128), f16)
        for b in range(NB):
            n0 = c * NPC + b * BLK
            R2[0, b, :BLK] = mcol_all[n0 : n0 + BLK]
            R2[1, b, :BLK] = 1.0
        in_maps.append(
            dict(
                shared,
                nfTo=np.ascontiguousarray(
                    node_feat.T[:, c * NPC : (c + 1) * NPC].astype(f16)
                ),
                R2=R2.reshape(2, NB * 128),
                gidx=_wrap_idx16(gidx),
                dstf=np.ascontiguousarray(dstl.reshape(T_tot, 128).T),
                rcolE=np.ascontiguousarray(rcole.reshape(T_tot, 128).T),
                ef=np.ascontiguousarray(
                    ef_rows.reshape(T_tot, 128, D).transpose(1, 0, 2)
                ),
            )
        )

    nc = _build_graph(T_list)
    res = run_bass_kernel_spmd(nc, in_maps, core_ids=list(range(C)))
    LAST_EXEC_NS = res.exec_time_ns
    LAST_RESULTS = res
    out = np.concatenate(
        [np.ascontiguousarray(res.results[c]["outT"].T) for c in range(C)], axis=0
    )
    return out


# revision 7
# speedup vs baseline: 1.4900x; 1.4900x over previous
"""BasicGNNConv on 8 TRN2 NeuronCores (Bass/Tile).

Math (reference):
    h   = node_feat @ Wn + bn                    # [N, 128]
    e   = edge_feat @ We + be                    # [E, 128]
    m   = h[src] + e
    agg = segment_sum(m, dst) / max(deg, 1)
    out = concat([h, agg]) @ Wc + bc

Linearity rewrite (eliminates all per-edge matmuls; biases folded):
    ht   = node_feat @ Wn                        # no bias
    S'   = onehot(dst) * rcol[dst]               # mean folded into the one-hot
    aggT = (S'h gathered-sum)T + We.T @ (S'ef sum)T          # [feat, slot]
    outT = Wc1.T @ htT_own + Wc2.T @ aggT + bnbeWc2 (x) mcol + bias0 (x) 1

Sharding: edges are assigned to the core that owns their dst node range
(5000 nodes/core) -> per-core segment sums are complete, no collective needed.

Per-core device pipeline:
  A.  ht (fp16) for all 40000 nodes in 2048-node chunks, written to two
      partition-major HBM tables (node n -> row (n%128)*TCOLS + n//128) so
      both the chunked writes and the per-edge gathers use efficient
      descriptors; gather indices are host-remapped to this layout.
  A2. ht.T for the core's own 5000 nodes (fp16, kept in SBUF).
  B.  Edge stream grouped by (superblock of 2 dst-blocks, src-half, block):
      gather ht[src] rows in up-to-24-tile SWDGE calls (64KB descriptor
      carveout -> 4096-desc rings), build the rcol-scaled one-hot S' with a
      single DVE tensor_scalar (is_equal then mult), and accumulate the
      TRANSPOSED segment sums acc_hT/acc_eT = [feat, slot] into one shared
      PSUM bank per block (lhsT = data tile, rhs = S').  The epilogue applies
      We to acc_eT (no transposes needed), then emits the output transposed;
      the host un-transposes.
"""
import numpy as np

import concourse.bacc as bacc
import concourse.mybir as mybir
import concourse.tile as tile
from concourse.tile_rust import add_dep_helper
from concourse.bass_utils import run_bass_kernel_spmd

N = 40000
E = 640000
D = 128          # OUT_DIM == EDGE_DIM
ND = 256         # NODE_DIM
C = 8            # cores
NPC = N // C     # 5000 nodes per core
BLK = 125        # nodes per dst block
NB = NPC // BLK  # 40 blocks per core
SBLK = 2         # blocks per superblock (PSUM-bounded)
NSB = NB // SBLK
NLO = 20480      # nodes in the lo gather table (10 phase-A chunks)
TLO = NLO // 128          # 160 t-columns
NHICAP = 19584            # 153 * 128 (capacity; real nodes 19520)
THI = NHICAP // 128       # 153
CH = 2048        # phase A chunk (nodes)
NCH = (N + CH - 1) // CH  # 20 (last chunk 1088 nodes)
CH2 = 500        # phase A2 chunk (own nodes)
PAD_COL = 127    # trash column in the 128-wide S window (>= BLK)
GMAX = 8         # tiles per dma_gather call
NQ = 4           # SWDGE queues
SCRATCH = 16384  # dynamic DMA scratch (default)

LAST_EXEC_NS = None
LAST_RESULTS = None

f16 = np.float16


def _wrap_idx16(arr):
    """[L] -> [128, L//16] int16 wrapped layout (pos i at [i%16, i//16]),
    replicated across the 8 GPSIMD core partition groups."""
    w = arr.astype(np.int16).reshape(-1, 16).T
    return np.ascontiguousarray(np.tile(w, (8, 1)))


def _build_graph(T_list):
    nc = bacc.Bacc(
        None, target_bir_lowering=False, debug=False,
        num_swdge_queues=NQ, dynamic_dma_scratch_size=SCRATCH,
    )
    f32, i16, fh = mybir.dt.float32, mybir.dt.int16, mybir.dt.float16

    T_tot = sum(tl + th for tl, th in T_list)
    L = T_tot * 128

    nfT_p = nc.declare_dram_parameter("nfT", [ND, N], fh, isOutput=False)
    nfTo_p = nc.declare_dram_parameter("nfTo", [ND, NPC], fh, isOutput=False)
    Wn_p = nc.declare_dram_parameter("Wn16", [ND, D], fh, isOutput=False)
    We_p = nc.declare_dram_parameter("We16", [D, D], fh, isOutput=False)
    Wc1_p = nc.declare_dram_parameter("Wc116", [D, D], fh, isOutput=False)
    Wc2_p = nc.declare_dram_parameter("Wc216", [D, D], fh, isOutput=False)
    L2_p = nc.declare_dram_parameter("L2", [2, D], fh, isOutput=False)
    R2_p = nc.declare_dram_parameter("R2", [2, NB * 128], fh, isOutput=False)
    gidx_p = nc.declare_dram_parameter("gidx", [128, L // 16], i16, isOutput=False)
    dstf_p = nc.declare_dram_parameter("dstf", [128, T_tot], fh, isOutput=False)
    rcolR_p = nc.declare_dram_parameter("rcolR", [128, NB * 128], f32, isOutput=False)
    ef_p = nc.declare_dram_parameter("ef", [128, T_tot, D], fh, isOutput=False)
    outT_p = nc.declare_dram_parameter("outT", [D, NPC], f32, isOutput=True)

    htab_lo = nc.dram_tensor("htab_lo", [NLO, D], fh)
    htab_hi = nc.dram_tensor("htab_hi", [NHICAP, D], fh)

    with tile.TileContext(nc) as tc:
        with (
            tc.tile_pool(name="const", bufs=1) as cpool,
            tc.tile_pool(name="tabs", bufs=1) as tpool,
        ):
            # ---- constants / weights in SBUF ----
            iota_i = cpool.tile([128, 128], mybir.dt.int32)
            nc.gpsimd.iota(iota_i[:], pattern=[[1, 128]], base=0, channel_multiplier=0)
            iota4 = cpool.tile([128, 4, 128], fh)
            for jj in range(4):
                nc.vector.tensor_copy(iota4[:, jj, :], iota_i[:])

            Wn_sb = cpool.tile([128, ND // 128, D], fh)
            nc.sync.dma_start(out=Wn_sb[:], in_=Wn_p[:].rearrange("(k p) d -> p k d", p=128))
            We_sb = cpool.tile([128, D], fh)
            nc.sync.dma_start(out=We_sb[:], in_=We_p[:])
            Wc1_sb = cpool.tile([128, D], fh)
            nc.sync.dma_start(out=Wc1_sb[:], in_=Wc1_p[:])
            Wc2_sb = cpool.tile([128, D], fh)
            nc.sync.dma_start(out=Wc2_sb[:], in_=Wc2_p[:])
            L2_sb = cpool.tile([2, D], fh)
            nc.sync.dma_start(out=L2_sb[:], in_=L2_p[:])
            R2_sb = cpool.tile([2, NB, 128], fh)
            nc.sync.dma_start(out=R2_sb[:], in_=R2_p[:].rearrange("p (b j) -> p b j", j=128))

            gidx_sb = cpool.tile([128, L // 16], i16)
            nc.sync.dma_start(out=gidx_sb[:], in_=gidx_p[:])
            dstf_sb = cpool.tile([128, T_tot], fh)
            nc.sync.dma_start(out=dstf_sb[:], in_=dstf_p[:])
            rcolR_sb = cpool.tile([128, NB, 128], f32)
            nc.sync.dma_start(out=rcolR_sb[:], in_=rcolR_p[:].rearrange("p (b j) -> p b j", j=128))

            hownT = tpool.tile([128, NB, BLK], fh)  # ht.T of own nodes

            # ---- Phase A: ht (fp16) -> partition-major htab tables ----
            with (
                tc.tile_pool(name="phA", bufs=3) as apool,
                tc.tile_pool(name="psA", bufs=2, space="PSUM") as apsum,
            ):
                last_htab_w = None
                for ci in range(NCH):
                    n0 = ci * CH
                    P = min(CH, N - n0)
                    nsub = (P + 127) // 128
                    nf_t = apool.tile([128, 2, CH], fh, tag="nf")
                    nc.sync.dma_start(
                        out=nf_t[:, :, :P],
                        in_=nfT_p[:, n0 : n0 + P].rearrange("(k p) n -> p k n", p=128),
                    )
                    hb = apool.tile([128, CH // 128, D], fh, tag="hb")
                    for g0 in range(0, nsub, 4):
                        gw = min(4, nsub - g0)
                        ps = apsum.tile([128, 4, D], f32, tag="psA")
                        for s in range(g0, g0 + gw):
                            sp = min(128, P - s * 128)
                            for k in range(2):
                                nc.tensor.matmul(
                                    ps[:sp, s - g0, :],
                                    lhsT=nf_t[:, k, s * 128 : s * 128 + sp],
                                    rhs=Wn_sb[:, k, :],
                                    start=(k == 0),
                                    stop=(k == 1),
                                )
                        nc.scalar.activation(
                            hb[:, g0 : g0 + gw, :], ps[:, :gw, :],
                            mybir.ActivationFunctionType.Copy,
                        )
                    if ci < NLO // CH:
                        dst_ap = htab_lo[:].rearrange("(p t) d -> p t d", p=128)[
                            :, ci * (CH // 128) : ci * (CH // 128) + nsub, :
                        ]
                    else:
                        t0 = (ci - NLO // CH) * (CH // 128)
                        dst_ap = htab_hi[:].rearrange("(p t) d -> p t d", p=128)[
                            :, t0 : t0 + nsub, :
                        ]
                    last_htab_w = nc.sync.dma_start(out=dst_ap, in_=hb[:, :nsub, :])

                # ---- Phase A2: ht.T of own nodes (fp16, transposed layout) ----
                for ci in range(NPC // CH2):
                    n0 = ci * CH2
                    nfo = apool.tile([128, 2, CH2], fh, tag="nfo")
                    d = nc.sync.dma_start(
                        out=nfo[:],
                        in_=nfTo_p[:, n0 : n0 + CH2].rearrange("(k p) n -> p k n", p=128),
                    )
                    add_dep_helper(d.ins, last_htab_w.ins, reason="defer A2 dma past htab")
                    ps2 = apsum.tile([128, 4, BLK], f32, tag="psA2")
                    for k in range(2):
                        nc.tensor.matmul(
                            ps2[:],
                            lhsT=Wn_sb[:, k, :],
                            rhs=nfo[:, k, :],
                            start=(k == 0),
                            stop=(k == 1),
                        )
                    nc.scalar.activation(
                        hownT[:, ci * 4 : ci * 4 + 4, :], ps2[:],
                        mybir.ActivationFunctionType.Copy,
                    )

            # ---- Phase B: edge stream + per-block combine epilogue ----
            sb_T = []
            for sb in range(NSB):
                blocks = [sb * SBLK + j for j in range(SBLK)]
                tlo = sum(T_list[b][0] for b in blocks)
                thi = sum(T_list[b][1] for b in blocks)
                sb_T.append((tlo, thi))
            TSBMAX = max(tl + th for tl, th in sb_T)

            with (
                tc.tile_pool(name="phB", bufs=3) as bpool,
                tc.tile_pool(name="phS", bufs=6) as spool,
                tc.tile_pool(name="phC", bufs=2) as cpl,
                tc.tile_pool(name="phO", bufs=2) as opool,
                tc.tile_pool(name="psB", bufs=2, space="PSUM") as bpsum,
            ):
                goff = 0
                qi = 0
                for sb in range(NSB):
                    blocks = [sb * SBLK + j for j in range(SBLK)]
                    Tlo_sb, Thi_sb = sb_T[sb]
                    Tsb = Tlo_sb + Thi_sb
                    eft = bpool.tile([128, TSBMAX, D], fh, tag="eft")
                    nc.sync.dma_start(out=eft[:, :Tsb, :], in_=ef_p[:, goff : goff + Tsb, :])
                    gl = bpool.tile([128, TSBMAX, D], fh, tag="gl")
                    for c0 in range(0, Tlo_sb, GMAX):
                        ch = min(GMAX, Tlo_sb - c0)
                        nc.gpsimd.dma_gather(
                            gl[:, c0 : c0 + ch, :],
                            htab_lo[:],
                            gidx_sb[:, (goff + c0) * 8 : (goff + c0 + ch) * 8],
                            ch * 128, ch * 128, D,
                            queue_num=qi % NQ,
                        )
                        qi += 1
                    for c0 in range(0, Thi_sb, GMAX):
                        ch = min(GMAX, Thi_sb - c0)
                        nc.gpsimd.dma_gather(
                            gl[:, Tlo_sb + c0 : Tlo_sb + c0 + ch, :],
                            htab_hi[:],
                            gidx_sb[:, (goff + Tlo_sb + c0) * 8 : (goff + Tlo_sb + c0 + ch) * 8],
                            ch * 128, ch * 128, D,
                            queue_num=qi % NQ,
                        )
                        qi += 1

                    accs = {}
                    started = {}
                    for j, b in enumerate(blocks):
                        accs[b] = bpsum.tile(
                            [128, 256], f32, tag=f"acc{j}", name=f"acc{j}_{sb}"
                        )
                        started[b] = False
                    # (block, last-flag) per tile position in stream order
                    tile_meta = []
                    for h in range(2):
                        for b in blocks:
                            tcount = T_list[b][h]
                            for u in range(tcount):
                                last = (u == tcount - 1) and (
                                    h == 1 or T_list[b][1] == 0
                                )
                                tile_meta.append((b, last))
                    for t0 in range(0, Tsb, 4):
                        w = min(4, Tsb - t0)
                        S4 = spool.tile([128, 4, 128], fh, tag="S", name="S4")
                        nc.vector.tensor_tensor(
                            out=S4[:, :w, :],
                            in0=dstf_sb[:, goff + t0 : goff + t0 + w, None].to_broadcast(
                                [128, w, 128]
                            ),
                            in1=iota4[:, :w, :],
                            op=mybir.AluOpType.is_equal,
                        )
                        for jj in range(w):
                            trel = t0 + jj
                            b, last = tile_meta[trel]
                            acc = accs[b]
                            first = not started[b]
                            started[b] = True
                            # acc_hT[f, slot] += gl_t.T @ S
                            nc.tensor.matmul(
                                acc[:, 0:128], lhsT=gl[:, trel, :], rhs=S4[:, jj, :],
                                start=first, stop=False, skip_group_check=True,
                            )
                            # acc_eT[f, slot] += eft_t.T @ S
                            # start=False always: MM_h's start=True above cleared
                            # the whole bank's has_written bits, so this group's
                            # first write is already "fresh"; a second start=True
                            # would wipe MM_h's tile.
                            nc.tensor.matmul(
                                acc[:, 128:256], lhsT=eft[:, trel, :], rhs=S4[:, jj, :],
                                start=False, stop=last, skip_group_check=True,
                            )
                    goff += Tsb

                    # ---- per-block epilogue: combine (all transposed) ----
                    osb = opool.tile([128, SBLK, BLK], f32, tag="osb")
                    for j, b in enumerate(blocks):
                        acc = accs[b]
                        SefT = cpl.tile([128, BLK], fh, tag="SefT")
                        nc.scalar.activation(
                            SefT[:], acc[:, 128 : 128 + BLK],
                            mybir.ActivationFunctionType.Copy,
                        )
                        # acc_hT[:, :125] += We.T @ SefT  (same open group)
                        nc.tensor.matmul(
                            acc[:, 0:BLK], lhsT=We_sb[:], rhs=SefT[:],
                            start=False, stop=True, skip_group_check=True,
                        )
                        aggT = cpl.tile([128, BLK], fh, tag="aggT")
                        nc.vector.tensor_tensor(
                            out=aggT[:], in0=acc[:, 0:BLK],
                            in1=rcolR_sb[:, b, :BLK],
                            op=mybir.AluOpType.mult,
                        )
                        poT = bpsum.tile([128, BLK], f32, tag="poT")
                        nc.tensor.matmul(poT[:], lhsT=Wc1_sb[:], rhs=hownT[:, b, :],
                                         start=True, stop=False)
                        nc.tensor.matmul(poT[:], lhsT=Wc2_sb[:], rhs=aggT[:],
                                         start=False, stop=False)
                        nc.tensor.matmul(poT[:], lhsT=L2_sb[:], rhs=R2_sb[:, b, :BLK],
                                         start=False, stop=True)
                        nc.vector.tensor_copy(osb[:, j, :], poT[:])
                    nc.sync.dma_start(
                        out=outT_p[:, sb * SBLK * BLK : (sb + 1) * SBLK * BLK],
                        in_=osb[:],
                    )

    nc.finalize()
    return nc


def kernel(node_feat, edge_feat, Wn, bn, We, be, Wc, bc, src, dst):
    global LAST_EXEC_NS, LAST_RESULTS
    node_feat = np.asarray(node_feat, np.float32)
    edge_feat = np.asarray(edge_feat, np.float32)
    Wn = np.asarray(Wn, np.float32)
    bn = np.asarray(bn, np.float32)
    We = np.asarray(We, np.float32)
    be = np.asarray(be, np.float32)
    Wc = np.asarray(Wc, np.float32)
    bc = np.asarray(bc, np.float32)
    src = np.asarray(src).astype(np.int64)
    dst = np.asarray(dst).astype(np.int64)

    # ---- host-side edge sharding / ordering ----
    cid = dst // NPC
    rel = dst - cid * NPC
    blk = rel // BLK
    dl = (rel - blk * BLK).astype(np.int64)
    sbi = blk // SBLK
    jin = blk - sbi * SBLK
    half = (src >= NLO).astype(np.int64)
    # stream order: (core, superblock, half, block-within-superblock)
    group = ((cid * NSB + sbi) * 2 + half) * SBLK + jin
    order = np.argsort(group, kind="stable")
    counts = np.bincount(group, minlength=C * NSB * 2 * SBLK).reshape(C, NSB, 2, SBLK)
    tcnt = (counts + 127) // 128
    Tmax = tcnt.max(axis=0)                      # [NSB, 2, SBLK]
    T_list = [
        (int(Tmax[b // SBLK, 0, b % SBLK]), int(Tmax[b // SBLK, 1, b % SBLK]))
        for b in range(NB)
    ]
    T_tot = int(Tmax.sum())
    L = T_tot * 128

    deg = np.bincount(dst, minlength=N).astype(np.float32)
    rcol_all = 1.0 / np.maximum(deg, 1.0)
    mcol_all = np.minimum(deg, 1.0)

    # remapped gather row index (partition-major table layout)
    gmap = np.where(
        src < NLO,
        (src % 128) * TLO + src // 128,
        ((src - NLO) % 128) * THI + (src - NLO) // 128,
    ).astype(np.int16)

    ef_h = edge_feat.astype(f16)

    # slot offsets in stream order
    gstart = np.zeros(C * NSB * 2 * SBLK + 1, np.int64)
    np.cumsum(counts.ravel(), out=gstart[1:])
    slot_off = np.zeros(NSB * 2 * SBLK + 1, np.int64)
    np.cumsum(Tmax.ravel() * 128, out=slot_off[1:])

    bnbeWc2 = (bn + be) @ Wc[D:]
    bias0 = bn @ Wc[:D] + bc
    L2 = np.stack([bnbeWc2, bias0]).astype(f16)

    in_maps = []
    shared = {
        "nfT": np.ascontiguousarray(node_feat.T.astype(f16)),
        "Wn16": Wn.astype(f16),
        "We16": We.astype(f16),
        "Wc116": np.ascontiguousarray(Wc[:D]).astype(f16),
        "Wc216": np.ascontiguousarray(Wc[D:]).astype(f16),
        "L2": L2,
    }
    for c in range(C):
        gidx = np.zeros(L, np.int16)
        dstl = np.full(L, PAD_COL, np.float16)
        eids = np.full(L, -1, np.int64)
        for g_local in range(NSB * 2 * SBLK):
            g = c * (NSB * 2 * SBLK) + g_local
            n = counts.ravel()[g]
            s0 = gstart[g]
            o0 = slot_off[g_local]
            ed = order[s0 : s0 + n]
            gidx[o0 : o0 + n] = gmap[ed]
            dstl[o0 : o0 + n] = dl[ed].astype(np.float16)
            eids[o0 : o0 + n] = ed
        ef_rows = np.zeros((L, D), f16)
        real = eids >= 0
        ef_rows[real] = ef_h[eids[real]]
        R2 = np.zeros((2, NB, 128), f16)
        rcolR = np.zeros((NB, 128), np.float32)
        for b in range(NB):
            n0 = c * NPC + b * BLK
            R2[0, b, :BLK] = mcol_all[n0 : n0 + BLK]
            R2[1, b, :BLK] = 1.0
            rcolR[b, :BLK] = rcol_all[n0 : n0 + BLK]
        rcolR_full = np.ascontiguousarray(
            np.broadcast_to(rcolR.reshape(1, NB * 128), (128, NB * 128))
        )
        in_maps.append(
            dict(
                shared,
                nfTo=np.ascontiguousarray(
                    node_feat.T[:, c * NPC : (c + 1) * NPC].astype(f16)
                ),
                R2=R2.reshape(2, NB * 128),
                rcolR=rcolR_full,
                gidx=_wrap_idx16(gidx),
                dstf=np.ascontiguousarray(dstl.reshape(T_tot, 128).T),
                ef=np.ascontiguousarray(
                    ef_rows.reshape(T_tot, 128, D).transpose(1, 0, 2)
                ),
            )
        )

    nc = _build_graph(T_list)
    res = run_bass_kernel_spmd(nc, in_maps, core_ids=list(range(C)))
    LAST_EXEC_NS = res.exec_time_ns
    LAST_RESULTS = res
    out = np.concatenate(
        [np.ascontiguousarray(res.results[c]["outT"].T) for c in range(C)], axis=0
    )
    return out
